# revision 20
# baseline (speedup 1.0000x reference)
"""Trainium2 Bass kernel for nn_MultiHeadAttention_39324720562623.

Reference computation (N=4, T=2048, D=512, H=8, HD=64), fp32:
    keys   = query @ Wk.T + query
    values = query @ Wv.T
    per head h: scores = softmax((Q_h @ K_h.T) / sqrt(HD))
    out = concat_heads(softmax @ V_h) @ Wo.T

Sharding: 8 cores = 4 batches x 2 head-groups (4 heads each), pure SPMD.
Each core computes K/V projections for its head-group only, attention for
its 4 heads, and a partial O-projection; the host sums the two partials
per batch.

The hard floor per core is the ACT (scalar) engine: 16.8M exps at
128 lanes / 1.2GHz = ~109us engine time that no other engine can take
(only ACT has the activation table).  The schedule therefore exists to
keep ACT saturated from ~7us on:

  - projections (K/V) are interleaved with the first q-tile's attention,
    paced by the chunked qT DMA, so exp work starts as soon as the first
    512 keys exist instead of after all projections.
  - heads are packed in pairs on the 128 partitions (features 64h..64h+63
    at rows 64*(h%2)..): scores contract over a 64-partition slice (no
    zero-padded kT), attT/woT pack 2 heads per 128 rows so the final
    projection runs at 2 matmuls per 128-out-dim chunk.
  - softmax normalization: denominator comes free as a ones-column
    appended to V (row 64 of att.T PSUM); 1/x via reciprocal_approx_fast
    (single custom-DVE op, ~5x faster than InstReciprocal, 18-bit); the
    row is broadcast across partitions with a tiny PE matmul.
  - input DMAs are issued from the idle Pool engine (weights) and SP
    (qT chunks) in need-order; output DMAs alternate SP/Pool queues.

All matmuls are float32r with moving free dim >= 256 (full PE rate).
exp needs no max-subtraction: scores/8 are bounded (|s|<~16).
"""

import os
import sys
from collections import deque

for _p in ("/opt/trn_rl_repo", os.path.join(os.path.expanduser("~"), ".axon_site", "_ro", "trn_rl_repo")):
    if os.path.isdir(_p) and _p not in sys.path:
        sys.path.insert(0, _p)
        break

import numpy as np

import concourse.bass as bass
import concourse.tile as tile
from concourse import mybir
from concourse.bass_utils import run_bass_kernel_spmd
from concourse.vector_clock import ScopedClock

N, T, D, H = 4, 2048, 512, 8
HD = D // H            # 64 head dim
HG = 2                 # head groups (cores per batch)
HPG = H // HG          # 4 heads per group
DG = D // HG           # 256 feature dims per group
F32 = mybir.dt.float32
F32R = mybir.dt.float32r
EXP = mybir.ActivationFunctionType.Exp
MULT = mybir.AluOpType.mult

QT = 512               # q-tile (matmul moving free dim)
NQT = T // QT          # 4
KC = 128               # k-chunk (partition dim of scores.T tiles)
NKC = T // KC          # 16
SGRP = 2               # k-chunks per scores-psum tile / exp instruction
NG = NKC // SGRP       # 8 score groups per (q-tile, head)
SCW = SGRP * QT        # 1024 scores tile free width


def _patch_drain():
    """walrus in this toolchain rejects >1 sync-wait on the kernel-tail
    Drain; split the waits across a chain of drains (1 wait each)."""
    if getattr(tile.TileContext, "_drain_split_patch", False):
        return

    def _drain_and_barrier(self, tick_clock, wait_clock):
        nc = self.nc
        d = nc.sync.drain()
        wait_clock.add_sem_waits(d.ins, ScopedClock({None: tick_clock.global_clock}))
        si = d.ins.sync_info
        waits = list(si.on_wait) if (si is not None and si.on_wait) else []
        if len(waits) > 1:
            si.on_wait = waits[:1]
            for w in waits[1:]:
                d2 = nc.sync.drain()
                if d2.ins.sync_info is None:
                    d2.ins.sync_info = mybir.SyncInfo(on_wait=[w], on_update=[])
                else:
                    d2.ins.sync_info.on_wait = [w]
        nc.all_engine_barrier()
        popped = nc._tile_sem_poison_stack.pop()
        assert popped is self._sem_poison
        nc.clear_and_free_semaphores(list(self.sems.allocated().values()))
        nc.all_engine_barrier()

    tile.TileContext._drain_and_barrier = _drain_and_barrier
    tile.TileContext._drain_split_patch = True


MAX_WAITS = 1


def _split_excess_waits(nc, maxw=MAX_WAITS):
    """walrus codegen rejects instructions with more than ~2 sync waits.
    Hoist excess waits onto same-engine nops inserted immediately before
    the offending instruction (same engine-stream position => identical
    semantics)."""
    nid = [0]

    def mk_nop(engine, waits):
        nid[0] += 1
        nop = mybir.InstNoOp(name=f"I-waitsplit-{nid[0]}")
        nop.engine = engine
        nop.sync_info = mybir.SyncInfo(on_wait=list(waits), on_update=[])
        try:
            nop.bass_nofuse = True
        except Exception:
            pass
        return nop

    for f in nc.m.functions:
        for bb in f.blocks:
            insts = bb.instructions
            i = 0
            while i < len(insts):
                ins = insts[i]
                si = ins.sync_info
                waits = list(si.on_wait) if (si is not None and si.on_wait) else []
                if len(waits) > maxw:
                    si.on_wait = waits[-maxw:]
                    excess = waits[:-maxw]
                    pos = i
                    for j in range(0, len(excess), maxw):
                        insts.insert(pos, mk_nop(ins.engine, excess[j : j + maxw]))
                        pos += 1
                        i += 1
                i += 1


def build_program():
    _patch_drain()
    nc = bass.Bass()
    qT = nc.dram_tensor("qT", [D, T], F32R, kind="ExternalInput")
    wkiT = nc.dram_tensor("wkiT", [D, DG], F32R, kind="ExternalInput")
    wvT = nc.dram_tensor("wvT", [D, DG], F32R, kind="ExternalInput")
    woT = nc.dram_tensor("woT", [DG, D], F32R, kind="ExternalInput")
    outT = nc.dram_tensor("outT", [D, T], F32, kind="ExternalOutput")

    with tile.TileContext(nc) as tc:
        with (
            tc.tile_pool(name="singles", bufs=1) as singles,
            tc.tile_pool(name="pt", bufs=16) as ptp,
            tc.tile_pool(name="recp", bufs=3) as recp,
            tc.tile_pool(name="outp", bufs=4) as outp,
            tc.tile_pool(name="ps_sc", bufs=2, space="PSUM") as ps_sc,
            tc.tile_pool(name="ps_att", bufs=2, space="PSUM") as ps_att,
            tc.tile_pool(name="ps_aux", bufs=2, space="PSUM") as ps_aux,
        ):
            qT_sb = singles.tile([128, 4, T], F32R)
            warm2 = singles.tile([1, 64], F32)
            wkiT_sb = singles.tile([128, 4, DG], F32R)
            wvT_sb = singles.tile([128, 4, DG], F32R)
            woT_sb = singles.tile([128, 2, D], F32R)
            sel_sb = singles.tile([128, 64], F32R)
            # two slots (pending-norm depth 2): row 64 of slot k%2 holds
            # head k's reciprocal row; all other rows stay zero
            rec_t = singles.tile([128, 2, QT], F32R)
            kT2 = singles.tile([128, 2, T], F32R)
            # V augmented with a ones column per head: [128, kchunk, head, 65]
            vaug_sb = singles.tile([128, NKC, HPG, HD + 1], F32R)
            attT_sb = singles.tile([128, 2, T], F32R)

            # ---- ACT table preload at t~0 ----
            nc.scalar.memzero(warm2[:])
            nc.scalar.activation(warm2[:], warm2[:], EXP)

            # ---- on-chip constants (idle engines, no DMA traffic) ----
            # (memset rejects f32r dtype; write through f32 views)
            nc.gpsimd.memset(sel_sb[:].bitcast(F32), 0.0)
            nc.vector.memset(sel_sb[64:65, :].bitcast(F32), 1.0)
            ones_sb = singles.tile([128, 64], F32R)
            nc.gpsimd.memset(ones_sb[:].bitcast(F32), 1.0)
            nc.vector.tensor_copy(
                vaug_sb[:, :, :, HD],
                ones_sb[:].rearrange("p (a b) -> p a b", b=HPG),
            )
            nc.vector.memset(rec_t[:].bitcast(F32), 0.0)

            # ---- input DMAs on SP, issued in need-order (Pool cannot issue
            # HWDGE; only SP/DVE/ACT can) ----
            qT_r2 = qT.rearrange("(c p) t -> p c t", p=128)
            nc.sync.dma_start(wkiT_sb[:], wkiT.rearrange("(c p) d -> p c d", p=128))
            nc.sync.dma_start(
                qT_sb[:, :, 0:QT], qT_r2[:, :, 0:QT]
            )
            nc.sync.dma_start(wvT_sb[:], wvT.rearrange("(c p) d -> p c d", p=128))
            for tt in range(1, 4):
                nc.sync.dma_start(
                    qT_sb[:, :, QT * tt : QT * (tt + 1)],
                    qT_r2[:, :, QT * tt : QT * (tt + 1)],
                )
            nc.sync.dma_start(woT_sb[:], woT.rearrange("(c p) d -> p c d", p=128))

            # ---------------- emission helpers ----------------
            def emit_kproj_dc(tt, dc):
                """kT chunk dc (features 128dc..) for keys QT*tt..QT*(tt+1)."""
                tsl = slice(QT * tt, QT * (tt + 1))
                ps = ps_aux.tile([128, QT], F32, tag="aux", name=f"kps{tt}_{dc}")
                for di in range(4):
                    nc.tensor.matmul(
                        ps[:],
                        wkiT_sb[:, di, 128 * dc : 128 * (dc + 1)],
                        qT_sb[:, di, tsl],
                        start=(di == 0),
                        stop=(di == 3),
                    )
                nc.vector.tensor_copy(kT2[:, dc, tsl], ps[:])

            def emit_vproj(tci):
                """V rows for tokens 128*tci.. (augmented layout)."""
                ps = ps_aux.tile([128, QT], F32, tag="aux", name=f"vps{tci}")
                for di in range(4):
                    nc.tensor.matmul(
                        ps[:, 0:DG],
                        qT_sb[:, di, 128 * tci : 128 * (tci + 1)],
                        wvT_sb[:, di],
                        start=(di == 0),
                        stop=(di == 3),
                    )
                nc.vector.tensor_copy(
                    vaug_sb[:, tci, :, 0:HD],
                    ps[:, 0:DG].rearrange("p (h d) -> p h d", d=HD),
                )

            def emit_sc_exp(jq, h, g):
                """scores + exp for k-chunks 2g,2g+1; returns the pt tile."""
                r0 = 64 * (h % 2)
                ch = h // 2
                qsl = slice(QT * jq, QT * (jq + 1))
                sc_ps = ps_sc.tile([128, SCW], F32, tag="sc", name=f"sc{jq}_{h}_{g}")
                for j in range(SGRP):
                    ik = SGRP * g + j
                    nc.tensor.matmul(
                        sc_ps[:, QT * j : QT * (j + 1)],
                        kT2[r0 : r0 + 64, ch, 128 * ik : 128 * (ik + 1)],
                        qT_sb[r0 : r0 + 64, ch, qsl],
                        start=True,
                        stop=True,
                    )
                pt = ptp.tile([128, SCW], F32R, tag="pt", name=f"pt{jq}_{h}_{g}")
                nc.scalar.activation(pt[:], sc_ps[:], EXP, scale=0.125)
                return pt

            att_state = {}  # global head k -> [att_ps, nmm]

            def emit_att(jq, h, g, pt):
                k = 4 * jq + h
                st = att_state.get(k)
                if st is None:
                    ap = ps_att.tile([128, QT], F32, tag="att", name=f"att{k}")
                    st = att_state[k] = [ap, 0]
                ap = st[0]
                for j in range(SGRP):
                    ik = SGRP * g + j
                    nc.tensor.matmul(
                        ap[0 : HD + 1, :],
                        vaug_sb[:, ik, h],
                        pt[:, QT * j : QT * (j + 1)],
                        start=(st[1] == 0),
                        stop=(st[1] == NKC - 1),
                    )
                    st[1] += 1

            pend_norms = deque()
            oq = deque()
            odma_ct = [0]

            def emit_norm_pending(jq, h):
                """issue the (cheap) reciprocal now; defer the rest."""
                k = 4 * jq + h
                ap = att_state[k][0]
                assert att_state[k][1] == NKC
                # issued ~a full head-iteration before its flush consumes it,
                # so the 3.4us DVE latency stays off every critical chain
                with nc.allow_low_precision(
                    reason="f32r recip output feeds the f32r broadcast matmul"
                ):
                    nc.vector.reciprocal(
                        rec_t[64:65, k % 2, :], ap[HD : HD + 1, :]
                    )
                pend_norms.append((jq, h))

            def flush_one():
                jq, h = pend_norms.popleft()
                k = 4 * jq + h
                ap = att_state.pop(k)[0]
                qsl = slice(QT * jq, QT * (jq + 1))
                rec_ps = ps_aux.tile([128, QT], F32, tag="aux", name=f"rp{k}")
                nc.tensor.matmul(
                    rec_ps[0:64, :],
                    sel_sb[:],
                    rec_t[:, k % 2, :],
                    start=True,
                    stop=True,
                )
                rec_bc = recp.tile([64, QT], F32R, tag="recbc", name=f"rb{k}")
                nc.vector.tensor_copy(rec_bc[:], rec_ps[0:64, :])
                nc.vector.tensor_tensor(
                    attT_sb[64 * (h % 2) : 64 * (h % 2) + 64, h // 2, qsl],
                    ap[0:HD, :],
                    rec_bc[:],
                    MULT,
                )
                if h == HPG - 1:
                    oq.extend((jq, dt) for dt in range(4))

            def emit_o(jq, dt, tail=False):
                qsl = slice(QT * jq, QT * (jq + 1))
                ops = ps_aux.tile([128, QT], F32, tag="aux", name=f"ops{jq}_{dt}")
                for c in range(2):
                    nc.tensor.matmul(
                        ops[:],
                        woT_sb[:, c, 128 * dt : 128 * (dt + 1)],
                        attT_sb[:, c, qsl],
                        start=(c == 0),
                        stop=(c == 1),
                    )
                ot = outp.tile([128, QT], F32, tag="ot", name=f"ot{jq}_{dt}")
                if tail and odma_ct[0] % 2 == 0:
                    nc.scalar.copy(ot[:], ops[:])
                else:
                    nc.vector.tensor_copy(ot[:], ops[:])
                # DVE can't issue DMAs; ACT only helps at the tail (idle then)
                eng = nc.scalar if (tail and odma_ct[0] % 2 == 1) else nc.sync
                eng.dma_start(outT[128 * dt : 128 * (dt + 1), qsl], ot[:])
                odma_ct[0] += 1

            # ---------------- Phase A: jq0 h0-h2 paced by qT DMA ----------------
            ptbuf = {}
            drained = {0: 0, 1: 0}  # head h -> groups att-drained so far

            def drain_att(h, upto):
                while drained[h] < upto:
                    g = drained[h]
                    emit_att(0, h, g, ptbuf.pop((h, g)))
                    drained[h] += 1
                    if drained[h] == NG:
                        emit_norm_pending(0, h)

            for tt in range(4):
                ga, gb = 2 * tt, 2 * tt + 1
                emit_kproj_dc(tt, 0)
                ptbuf[(0, ga)] = emit_sc_exp(0, 0, ga)
                emit_vproj(4 * tt + 0)
                ptbuf[(0, gb)] = emit_sc_exp(0, 0, gb)
                emit_vproj(4 * tt + 1)
                drain_att(0, ga)  # lag 2: drain up to previous window's groups
                emit_kproj_dc(tt, 1)
                ptbuf[(1, ga)] = emit_sc_exp(0, 1, ga)
                emit_vproj(4 * tt + 2)
                ptbuf[(1, gb)] = emit_sc_exp(0, 1, gb)
                emit_vproj(4 * tt + 3)
                drain_att(1, ga)
                ptbuf[(2, ga)] = emit_sc_exp(0, 2, ga)
                ptbuf[(2, gb)] = emit_sc_exp(0, 2, gb)
                drain_att(0, gb)
            drain_att(0, NG)
            drain_att(1, NG)

            # ---------------- Phase A2: jq0 h3 sc/exp + h2/h3 att drains ------
            flush_one()  # norm(0,0) -> frees bank for h2
            for g in range(NG):
                emit_att(0, 2, g, ptbuf.pop((2, g)))
                ptbuf[(3, g)] = emit_sc_exp(0, 3, g)
                if g == 1:
                    flush_one()  # norm(0,1) -> frees bank for h3
                if g >= 2:
                    emit_att(0, 3, g - 2, ptbuf.pop((3, g - 2)))
            emit_norm_pending(0, 2)
            for g in range(NG - 2, NG):
                emit_att(0, 3, g, ptbuf.pop((3, g)))
            emit_norm_pending(0, 3)
            assert not ptbuf

            # ---------------- Phase B: steady (jq 1..3) x heads ----------------
            for jq in range(1, NQT):
                for h in range(HPG):
                    for g in range(NG):
                        pt = emit_sc_exp(jq, h, g)
                        if g == 0:
                            flush_one()
                        if g == 4 and oq:
                            emit_o(*oq.popleft())
                        if g == 6 and oq:
                            emit_o(*oq.popleft())
                        emit_att(jq, h, g, pt)
                    emit_norm_pending(jq, h)

            # ---------------- tail ----------------
            flush_one()
            flush_one()
            while oq:
                emit_o(*oq.popleft(), tail=True)
            assert not pend_norms and not att_state

    _split_excess_waits(nc)
    return nc


_CACHED_NC = None


def _get_nc():
    global _CACHED_NC
    if _CACHED_NC is None:
        _CACHED_NC = build_program()
    return _CACHED_NC


def _shard_inputs(query, Wk, Wv, Wo):
    wki = Wk.astype(np.float32) + np.eye(D, dtype=np.float32)
    in_maps = []
    perms = []
    for g in range(HG):
        perm = np.r_[DG * g : DG * (g + 1), 0 : DG * g, DG * (g + 1) : D]
        perms.append(perm)
    for n in range(N):
        for g in range(HG):
            perm = perms[g]
            hg = slice(DG * g, DG * (g + 1))
            qTn = np.ascontiguousarray(query[n].T[perm])          # [512, 2048]
            wkiT = np.ascontiguousarray(wki[hg, :][:, perm].T)    # [512, 256]
            wvT = np.ascontiguousarray(Wv[hg, :][:, perm].T)      # [512, 256]
            woT = np.ascontiguousarray(Wo[:, hg].T)               # [256, 512]
            in_maps.append(
                {
                    "qT": qTn.astype(np.float32),
                    "wkiT": wkiT.astype(np.float32),
                    "wvT": wvT.astype(np.float32),
                    "woT": woT.astype(np.float32),
                }
            )
    return in_maps


def run(query, Wk, Wv, Wo, **run_kwargs):
    """Run the SPMD kernel; returns (output, BassKernelResults)."""
    nc = _get_nc()
    in_maps = _shard_inputs(
        np.asarray(query, dtype=np.float32),
        np.asarray(Wk, dtype=np.float32),
        np.asarray(Wv, dtype=np.float32),
        np.asarray(Wo, dtype=np.float32),
    )
    res = run_bass_kernel_spmd(nc, in_maps, list(range(N * HG)), **run_kwargs)
    outs = []
    for n in range(N):
        pT = res.results[2 * n]["outT"] + res.results[2 * n + 1]["outT"]
        outs.append(pT.T)
    return np.stack(outs).astype(np.float32), res


def kernel(query, Wk, Wv, Wo):
    out, _ = run(query, Wk, Wv, Wo)
    return out


# revision 24
# speedup vs baseline: 1.4114x; 1.4114x over previous
"""Trainium2 Bass kernel for nn_MultiHeadAttention_39324720562623.

Reference computation (N=4, T=2048, D=512, H=8, HD=64), fp32:
    keys   = query @ Wk.T + query
    values = query @ Wv.T
    per head h: scores = softmax((Q_h @ K_h.T) / sqrt(HD))
    out = concat_heads(softmax @ V_h) @ Wo.T

Sharding: 8 cores = 4 batches x 2 head-groups (4 heads each), pure SPMD.
Each core computes K/V projections for its head-group only, attention for
its 4 heads, and a partial O-projection; the host sums the two partials
per batch.

The hard floor per core is the ACT (scalar) engine: 16.8M exps at
128 lanes / 1.2GHz = ~109us engine time that no other engine can take
(only ACT has the activation table).  The schedule therefore exists to
keep ACT saturated from ~7us on:

  - projections (K/V) are interleaved with the first q-tile's attention,
    paced by the chunked qT DMA, so exp work starts as soon as the first
    512 keys exist instead of after all projections.
  - heads are packed in pairs on the 128 partitions (features 64h..64h+63
    at rows 64*(h%2)..): scores contract over a 64-partition slice (no
    zero-padded kT), attT/woT pack 2 heads per 128 rows so the final
    projection runs at 2 matmuls per 128-out-dim chunk.
  - softmax normalization: denominator comes free as a ones-column
    appended to V (row 64 of att.T PSUM); 1/x via reciprocal_approx_fast
    (single custom-DVE op, ~5x faster than InstReciprocal, 18-bit); the
    row is broadcast across partitions with a tiny PE matmul.
  - input DMAs are issued from the idle Pool engine (weights) and SP
    (qT chunks) in need-order; output DMAs alternate SP/Pool queues.

All matmuls are float32r with moving free dim >= 256 (full PE rate).
exp needs no max-subtraction: scores/8 are bounded (|s|<~16).
"""

import os
import sys
from collections import deque

for _p in ("/opt/trn_rl_repo", os.path.join(os.path.expanduser("~"), ".axon_site", "_ro", "trn_rl_repo")):
    if os.path.isdir(_p) and _p not in sys.path:
        sys.path.insert(0, _p)
        break

import numpy as np

import concourse.bass as bass
import concourse.tile as tile
from concourse import mybir
from concourse.bass_utils import run_bass_kernel_spmd
from concourse.vector_clock import ScopedClock

N, T, D, H = 4, 2048, 512, 8
HD = D // H            # 64 head dim
HG = 2                 # head groups (cores per batch)
HPG = H // HG          # 4 heads per group
DG = D // HG           # 256 feature dims per group
F32 = mybir.dt.float32
F32R = mybir.dt.float32r
EXP = mybir.ActivationFunctionType.Exp
MULT = mybir.AluOpType.mult

QT = 512               # q-tile (matmul moving free dim)
NQT = T // QT          # 4
KC = 128               # k-chunk (partition dim of scores.T tiles)
NKC = T // KC          # 16
SGRP = 2               # k-chunks per scores-psum tile / exp instruction
NG = NKC // SGRP       # 8 score groups per (q-tile, head)
SCW = SGRP * QT        # 1024 scores tile free width


def _patch_drain():
    """walrus in this toolchain rejects >1 sync-wait on the kernel-tail
    Drain; split the waits across a chain of drains (1 wait each)."""
    if getattr(tile.TileContext, "_drain_split_patch", False):
        return

    def _drain_and_barrier(self, tick_clock, wait_clock):
        nc = self.nc
        d = nc.sync.drain()
        wait_clock.add_sem_waits(d.ins, ScopedClock({None: tick_clock.global_clock}))
        si = d.ins.sync_info
        waits = list(si.on_wait) if (si is not None and si.on_wait) else []
        if len(waits) > 1:
            si.on_wait = waits[:1]
            for w in waits[1:]:
                d2 = nc.sync.drain()
                if d2.ins.sync_info is None:
                    d2.ins.sync_info = mybir.SyncInfo(on_wait=[w], on_update=[])
                else:
                    d2.ins.sync_info.on_wait = [w]
        nc.all_engine_barrier()
        popped = nc._tile_sem_poison_stack.pop()
        assert popped is self._sem_poison
        nc.clear_and_free_semaphores(list(self.sems.allocated().values()))
        nc.all_engine_barrier()

    tile.TileContext._drain_and_barrier = _drain_and_barrier
    tile.TileContext._drain_split_patch = True


MAX_WAITS = 1


def _split_excess_waits(nc, maxw=MAX_WAITS):
    """walrus codegen rejects instructions with more than ~2 sync waits.
    Hoist excess waits onto same-engine nops inserted immediately before
    the offending instruction (same engine-stream position => identical
    semantics)."""
    nid = [0]

    def mk_nop(engine, waits):
        nid[0] += 1
        nop = mybir.InstNoOp(name=f"I-waitsplit-{nid[0]}")
        nop.engine = engine
        nop.sync_info = mybir.SyncInfo(on_wait=list(waits), on_update=[])
        try:
            nop.bass_nofuse = True
        except Exception:
            pass
        return nop

    for f in nc.m.functions:
        for bb in f.blocks:
            insts = bb.instructions
            i = 0
            while i < len(insts):
                ins = insts[i]
                si = ins.sync_info
                waits = list(si.on_wait) if (si is not None and si.on_wait) else []
                if len(waits) > maxw:
                    si.on_wait = waits[-maxw:]
                    excess = waits[:-maxw]
                    pos = i
                    for j in range(0, len(excess), maxw):
                        insts.insert(pos, mk_nop(ins.engine, excess[j : j + maxw]))
                        pos += 1
                        i += 1
                i += 1


def build_program():
    _patch_drain()
    nc = bass.Bass()
    qT = nc.dram_tensor("qT", [D, T], F32R, kind="ExternalInput")
    wkiT = nc.dram_tensor("wkiT", [D, DG], F32R, kind="ExternalInput")
    wvT = nc.dram_tensor("wvT", [D, DG], F32R, kind="ExternalInput")
    woT = nc.dram_tensor("woT", [DG, D], F32R, kind="ExternalInput")
    outT = nc.dram_tensor("outT", [D, T], F32, kind="ExternalOutput")

    with tile.TileContext(nc) as tc:
        with (
            tc.tile_pool(name="singles", bufs=1) as singles,
            tc.tile_pool(name="pt", bufs=16) as ptp,
            tc.tile_pool(name="recp", bufs=3) as recp,
            tc.tile_pool(name="outp", bufs=4) as outp,
            tc.tile_pool(name="ps_sc", bufs=2, space="PSUM") as ps_sc,
            tc.tile_pool(name="ps_att", bufs=2, space="PSUM") as ps_att,
            tc.tile_pool(name="ps_aux", bufs=2, space="PSUM") as ps_aux,
        ):
            qT_sb = singles.tile([128, 4, T], F32R)
            warm2 = singles.tile([1, 64], F32)
            wkiT_sb = singles.tile([128, 4, DG], F32R)
            wvT_sb = singles.tile([128, 4, DG], F32R)
            woT_sb = singles.tile([128, 2, D], F32R)
            sel_sb = singles.tile([128, 64], F32R)
            # two slots (pending-norm depth 2): row 64 of slot k%2 holds
            # head k's reciprocal row; all other rows stay zero
            rec_t = singles.tile([128, 2, QT], F32R)
            # per-head K.T slots, zero-padded to the full 128 partitions:
            # 64-deep matmuls drop the PE HAM to K=4/8, which trips the
            # hardware's 50% util throttle (see baseline junk_mm comment) --
            # so scores MUST contract all 128 partitions.
            kT_pad = singles.tile([128, HPG, T], F32R)
            # V augmented with a ones column per head: [128, kchunk, head, 65]
            vaug_sb = singles.tile([128, NKC, HPG, HD + 1], F32R)
            attT_sb = singles.tile([128, 2, T], F32R)

            # ---- ACT table preload at t~0 ----
            nc.scalar.memzero(warm2[:])
            nc.scalar.activation(warm2[:], warm2[:], EXP)

            # ---- on-chip constants (idle engines, no DMA traffic) ----
            # (memset rejects f32r dtype; write through f32 views)
            nc.gpsimd.memset(sel_sb[:].bitcast(F32), 0.0)
            nc.vector.memset(sel_sb[64:65, :].bitcast(F32), 1.0)
            ones_sb = singles.tile([128, 64], F32R)
            nc.gpsimd.memset(ones_sb[:].bitcast(F32), 1.0)
            nc.vector.tensor_copy(
                vaug_sb[:, :, :, HD],
                ones_sb[:].rearrange("p (a b) -> p a b", b=HPG),
            )
            nc.vector.memset(rec_t[:].bitcast(F32), 0.0)
            for h in range(HPG):
                off = 64 - (h % 2) * 64  # complement of the head's parity slot
                nc.gpsimd.memset(kT_pad[off : off + 64, h, :].bitcast(F32), 0.0)

            # ---- input DMAs on SP, issued in need-order (Pool cannot issue
            # HWDGE; only SP/DVE/ACT can) ----
            qT_r2 = qT.rearrange("(c p) t -> p c t", p=128)
            nc.sync.dma_start(wkiT_sb[:], wkiT.rearrange("(c p) d -> p c d", p=128))
            nc.sync.dma_start(
                qT_sb[:, :, 0:QT], qT_r2[:, :, 0:QT]
            )
            nc.sync.dma_start(wvT_sb[:], wvT.rearrange("(c p) d -> p c d", p=128))
            for tt in range(1, 4):
                nc.sync.dma_start(
                    qT_sb[:, :, QT * tt : QT * (tt + 1)],
                    qT_r2[:, :, QT * tt : QT * (tt + 1)],
                )
            nc.sync.dma_start(woT_sb[:], woT.rearrange("(c p) d -> p c d", p=128))

            # ---------------- emission helpers ----------------
            def emit_kproj_dc(tt, dc):
                """kT chunk dc (features 128dc..) for keys QT*tt..QT*(tt+1)."""
                tsl = slice(QT * tt, QT * (tt + 1))
                ps = ps_aux.tile([128, QT], F32, tag="aux", name=f"kps{tt}_{dc}")
                for di in range(4):
                    nc.tensor.matmul(
                        ps[:],
                        wkiT_sb[:, di, 128 * dc : 128 * (dc + 1)],
                        qT_sb[:, di, tsl],
                        start=(di == 0),
                        stop=(di == 3),
                    )
                nc.vector.tensor_copy(kT_pad[0:64, 2 * dc, tsl], ps[0:64, :])
                nc.vector.tensor_copy(
                    kT_pad[64:128, 2 * dc + 1, tsl], ps[64:128, :]
                )

            def emit_vproj(tci):
                """V rows for tokens 128*tci.. (augmented layout)."""
                ps = ps_aux.tile([128, QT], F32, tag="aux", name=f"vps{tci}")
                for di in range(4):
                    nc.tensor.matmul(
                        ps[:, 0:DG],
                        qT_sb[:, di, 128 * tci : 128 * (tci + 1)],
                        wvT_sb[:, di],
                        start=(di == 0),
                        stop=(di == 3),
                    )
                nc.vector.tensor_copy(
                    vaug_sb[:, tci, :, 0:HD],
                    ps[:, 0:DG].rearrange("p (h d) -> p h d", d=HD),
                )

            def emit_sc_exp(jq, h, g):
                """scores + exp for k-chunks 2g,2g+1; returns the pt tile."""
                ch = h // 2
                qsl = slice(QT * jq, QT * (jq + 1))
                sc_ps = ps_sc.tile([128, SCW], F32, tag="sc", name=f"sc{jq}_{h}_{g}")
                for j in range(SGRP):
                    ik = SGRP * g + j
                    nc.tensor.matmul(
                        sc_ps[:, QT * j : QT * (j + 1)],
                        kT_pad[:, h, 128 * ik : 128 * (ik + 1)],
                        qT_sb[:, ch, qsl],
                        start=True,
                        stop=True,
                    )
                pt = ptp.tile([128, SCW], F32R, tag="pt", name=f"pt{jq}_{h}_{g}")
                nc.scalar.activation(pt[:], sc_ps[:], EXP, scale=0.125)
                return pt

            att_state = {}  # global head k -> [att_ps, nmm]

            def emit_att(jq, h, g, pt):
                k = 4 * jq + h
                st = att_state.get(k)
                if st is None:
                    ap = ps_att.tile([128, QT], F32, tag="att", name=f"att{k}")
                    st = att_state[k] = [ap, 0]
                ap = st[0]
                for j in range(SGRP):
                    ik = SGRP * g + j
                    nc.tensor.matmul(
                        ap[0 : HD + 1, :],
                        vaug_sb[:, ik, h],
                        pt[:, QT * j : QT * (j + 1)],
                        start=(st[1] == 0),
                        stop=(st[1] == NKC - 1),
                    )
                    st[1] += 1

            pend_norms = deque()
            oq = deque()
            odma_ct = [0]

            def emit_norm_pending(jq, h):
                """issue the (cheap) reciprocal now; defer the rest."""
                k = 4 * jq + h
                ap = att_state[k][0]
                assert att_state[k][1] == NKC
                # issued ~a full head-iteration before its flush consumes it,
                # so the 3.4us DVE latency stays off every critical chain
                with nc.allow_low_precision(
                    reason="f32r recip output feeds the f32r broadcast matmul"
                ):
                    nc.vector.reciprocal(
                        rec_t[64:65, k % 2, :], ap[HD : HD + 1, :]
                    )
                pend_norms.append((jq, h))

            def flush_one():
                jq, h = pend_norms.popleft()
                k = 4 * jq + h
                ap = att_state.pop(k)[0]
                qsl = slice(QT * jq, QT * (jq + 1))
                rec_ps = ps_aux.tile([128, QT], F32, tag="aux", name=f"rp{k}")
                nc.tensor.matmul(
                    rec_ps[0:64, :],
                    sel_sb[:],
                    rec_t[:, k % 2, :],
                    start=True,
                    stop=True,
                )
                rec_bc = recp.tile([64, QT], F32R, tag="recbc", name=f"rb{k}")
                nc.vector.tensor_copy(rec_bc[:], rec_ps[0:64, :])
                nc.vector.tensor_tensor(
                    attT_sb[64 * (h % 2) : 64 * (h % 2) + 64, h // 2, qsl],
                    ap[0:HD, :],
                    rec_bc[:],
                    MULT,
                )
                if h == HPG - 1:
                    oq.extend((jq, dt) for dt in range(4))

            def emit_o(jq, dt, tail=False):
                qsl = slice(QT * jq, QT * (jq + 1))
                ops = ps_aux.tile([128, QT], F32, tag="aux", name=f"ops{jq}_{dt}")
                for c in range(2):
                    nc.tensor.matmul(
                        ops[:],
                        woT_sb[:, c, 128 * dt : 128 * (dt + 1)],
                        attT_sb[:, c, qsl],
                        start=(c == 0),
                        stop=(c == 1),
                    )
                ot = outp.tile([128, QT], F32, tag="ot", name=f"ot{jq}_{dt}")
                if tail and odma_ct[0] % 2 == 0:
                    nc.scalar.copy(ot[:], ops[:])
                else:
                    nc.vector.tensor_copy(ot[:], ops[:])
                # DVE can't issue DMAs; ACT only helps at the tail (idle then)
                eng = nc.scalar if (tail and odma_ct[0] % 2 == 1) else nc.sync
                eng.dma_start(outT[128 * dt : 128 * (dt + 1), qsl], ot[:])
                odma_ct[0] += 1

            # ---------------- Phase A: jq0 h0-h2 paced by qT DMA ----------------
            ptbuf = {}
            drained = {0: 0, 1: 0}  # head h -> groups att-drained so far

            def drain_att(h, upto):
                while drained[h] < upto:
                    g = drained[h]
                    emit_att(0, h, g, ptbuf.pop((h, g)))
                    drained[h] += 1
                    if drained[h] == NG:
                        emit_norm_pending(0, h)

            for tt in range(4):
                ga, gb = 2 * tt, 2 * tt + 1
                emit_kproj_dc(tt, 0)
                ptbuf[(0, ga)] = emit_sc_exp(0, 0, ga)
                emit_vproj(4 * tt + 0)
                ptbuf[(0, gb)] = emit_sc_exp(0, 0, gb)
                emit_vproj(4 * tt + 1)
                drain_att(0, ga)  # lag 2: drain up to previous window's groups
                emit_kproj_dc(tt, 1)
                ptbuf[(1, ga)] = emit_sc_exp(0, 1, ga)
                emit_vproj(4 * tt + 2)
                ptbuf[(1, gb)] = emit_sc_exp(0, 1, gb)
                emit_vproj(4 * tt + 3)
                drain_att(1, ga)
                ptbuf[(2, ga)] = emit_sc_exp(0, 2, ga)
                ptbuf[(2, gb)] = emit_sc_exp(0, 2, gb)
                drain_att(0, gb)
            drain_att(0, NG)
            drain_att(1, NG)

            # ---------------- Phase A2: jq0 h3 sc/exp + h2/h3 att drains ------
            flush_one()  # norm(0,0) -> frees bank for h2
            for g in range(NG):
                emit_att(0, 2, g, ptbuf.pop((2, g)))
                ptbuf[(3, g)] = emit_sc_exp(0, 3, g)
                if g == 1:
                    flush_one()  # norm(0,1) -> frees bank for h3
                if g >= 2:
                    emit_att(0, 3, g - 2, ptbuf.pop((3, g - 2)))
            emit_norm_pending(0, 2)
            for g in range(NG - 2, NG):
                emit_att(0, 3, g, ptbuf.pop((3, g)))
            emit_norm_pending(0, 3)
            assert not ptbuf

            # ---------------- Phase B: steady (jq 1..3) x heads ----------------
            for jq in range(1, NQT):
                for h in range(HPG):
                    for g in range(NG):
                        pt = emit_sc_exp(jq, h, g)
                        if g == 0:
                            flush_one()
                        if g == 4 and oq:
                            emit_o(*oq.popleft())
                        if g == 6 and oq:
                            emit_o(*oq.popleft())
                        emit_att(jq, h, g, pt)
                    emit_norm_pending(jq, h)

            # ---------------- tail ----------------
            flush_one()
            flush_one()
            while oq:
                emit_o(*oq.popleft(), tail=True)
            assert not pend_norms and not att_state

    _split_excess_waits(nc)
    return nc


_CACHED_NC = None


def _get_nc():
    global _CACHED_NC
    if _CACHED_NC is None:
        _CACHED_NC = build_program()
    return _CACHED_NC


def _shard_inputs(query, Wk, Wv, Wo):
    wki = Wk.astype(np.float32) + np.eye(D, dtype=np.float32)
    in_maps = []
    perms = []
    for g in range(HG):
        perm = np.r_[DG * g : DG * (g + 1), 0 : DG * g, DG * (g + 1) : D]
        perms.append(perm)
    for n in range(N):
        for g in range(HG):
            perm = perms[g]
            hg = slice(DG * g, DG * (g + 1))
            qTn = np.ascontiguousarray(query[n].T[perm])          # [512, 2048]
            wkiT = np.ascontiguousarray(wki[hg, :][:, perm].T)    # [512, 256]
            wvT = np.ascontiguousarray(Wv[hg, :][:, perm].T)      # [512, 256]
            woT = np.ascontiguousarray(Wo[:, hg].T)               # [256, 512]
            in_maps.append(
                {
                    "qT": qTn.astype(np.float32),
                    "wkiT": wkiT.astype(np.float32),
                    "wvT": wvT.astype(np.float32),
                    "woT": woT.astype(np.float32),
                }
            )
    return in_maps


def run(query, Wk, Wv, Wo, **run_kwargs):
    """Run the SPMD kernel; returns (output, BassKernelResults)."""
    nc = _get_nc()
    in_maps = _shard_inputs(
        np.asarray(query, dtype=np.float32),
        np.asarray(Wk, dtype=np.float32),
        np.asarray(Wv, dtype=np.float32),
        np.asarray(Wo, dtype=np.float32),
    )
    res = run_bass_kernel_spmd(nc, in_maps, list(range(N * HG)), **run_kwargs)
    outs = []
    for n in range(N):
        pT = res.results[2 * n]["outT"] + res.results[2 * n + 1]["outT"]
        outs.append(pT.T)
    return np.stack(outs).astype(np.float32), res


def kernel(query, Wk, Wv, Wo):
    out, _ = run(query, Wk, Wv, Wo)
    return out
